# revision 1
# baseline (speedup 1.0000x reference)
"""GPTQ 4-bit quant linear (nn_Autograd4bitQuantLinear) on 8 TRN2 NeuronCores.

Strategy (column-parallel tensor parallelism, per sharding hint):
 - Host: dequantize packed 4-bit weights to W [4096, 11008] f32, shard along
   out_features (1376 per core). x transposed to xT [4096, 8192] (contraction
   on partitions), replicated across cores.
 - Device (per core): xT.T @ W_shard on the PE, fp32 PSUM accumulation.
   W shard stays resident in SBUF; x streams in 512-token blocks; psum chunks
   of 512/512/352 out-features; out [8192, 1376] f32 written back.
 - Host: concatenate the 8 shards along the last dim.

Precision plan: the contraction (32 k-tiles of 128) is split between
full-precision (fp16/bf16, 1 cyc/row on PE) k-tiles and fp8-e4m3 DoubleRow
k-tile PAIRS (2 MACs/cell/cycle, ~0.57 cyc/row-pair). Pure fp8 measures
rel err 3.73e-2 (> 2e-2 gate); err scales as sqrt(n_fp8_ktiles/32), so
N8 pairs add err 3.73e-2*sqrt(N8/16). Scales: x*32 and W*512 (exact powers
of two, chosen so fp8 values stay in [-240, 240], the TRN e4m3 range),
descaled by 1/16384 at PSUM eviction.
"""

import os
import numpy as np
import ml_dtypes

IN_F = 4096
OUT_F = 11008
GROUP = 128
TOKENS = 8192
NCORES = 8
SHARD = OUT_F // NCORES  # 1376
P = 128
KT = IN_F // P  # 32 k-tiles
TB = 512  # tokens per block
NBLK = TOKENS // TB  # 16
TSUB = TB // P  # 4
CHUNKS = [(0, 512), (512, 512), (1024, SHARD - 1024)]  # psum-bank sized chunks

# ---- precision configuration ----
MM_DT = "float16"  # dtype of the full-precision k-tiles: float16 or bfloat16
N8 = 0  # number of fp8 DoubleRow k-tile pairs (each covers 2 of the 32 k-tiles)
SX = 32.0  # x pre-scale (power of 2; applied when N8 > 0)
SW = 512.0  # W pre-scale (power of 2; applied when N8 > 0)

_CACHE = {}


def _cfg():
    return (MM_DT, N8)


def _build_nc(reps=1):
    import concourse.bass as bass
    import concourse.mybir as mybir
    import concourse.tile as tile
    from concourse import bacc

    kt16 = KT - 2 * N8  # full-precision k-tiles
    scale = (SX * SW) if N8 > 0 else 1.0

    nc = bacc.Bacc(
        "TRN2",
        target_bir_lowering=False,
        debug=False,
        enable_asserts=False,
        num_devices=NCORES,
    )
    mdt = getattr(mybir.dt, MM_DT)
    f8 = mybir.dt.float8e4
    f32 = mybir.dt.float32
    DR = mybir.MatmulPerfMode.DoubleRow

    xt16 = w16 = xt8 = w8 = None
    if kt16 > 0:
        xt16 = nc.dram_tensor("xt16", [kt16 * P, TOKENS], mdt, kind="ExternalInput").ap()
        w16 = nc.dram_tensor("w16", [kt16 * P, SHARD], mdt, kind="ExternalInput").ap()
    if N8 > 0:
        xt8 = nc.dram_tensor("xt8", [2 * N8 * P, TOKENS], f8, kind="ExternalInput").ap()
        w8 = nc.dram_tensor("w8", [2 * N8 * P, SHARD], f8, kind="ExternalInput").ap()
    out = nc.dram_tensor("out", [TOKENS, SHARD], f32, kind="ExternalOutput").ap()

    with tile.TileContext(nc) as tc:
        with (
            tc.tile_pool(name="wp", bufs=1) as wp,
            tc.tile_pool(name="xp", bufs=2) as xp,
            tc.tile_pool(name="op", bufs=2) as op,
            tc.tile_pool(name="pp", bufs=2, space=bass.MemorySpace.PSUM) as pp,
        ):
            if kt16 > 0:
                w16_sb = wp.tile([P, kt16, SHARD], mdt)
                for k in range(kt16):
                    nc.sync.dma_start(w16_sb[:, k, :], w16[k * P : (k + 1) * P, :])
            if N8 > 0:
                w8_sb = wp.tile([P, 2 * N8, SHARD], f8)
                for k in range(2 * N8):
                    nc.sync.dma_start(w8_sb[:, k, :], w8[k * P : (k + 1) * P, :])
            for _rep in range(reps):
                for b in range(NBLK):
                    if kt16 > 0:
                        x16_sb = xp.tile([P, kt16, TB], mdt, name="x16")
                        for k in range(kt16):
                            nc.sync.dma_start(
                                x16_sb[:, k, :],
                                xt16[k * P : (k + 1) * P, b * TB : (b + 1) * TB],
                            )
                    if N8 > 0:
                        x8_sb = xp.tile([P, 2 * N8, TB], f8, name="x8")
                        for k in range(2 * N8):
                            nc.sync.dma_start(
                                x8_sb[:, k, :],
                                xt8[k * P : (k + 1) * P, b * TB : (b + 1) * TB],
                            )
                    for s in range(TSUB):
                        o_sb = op.tile([P, SHARD], f32, name="o_sb")
                        pss = [
                            pp.tile([P, 512], f32, tag=f"ps{ci}", name=f"ps{ci}")
                            for ci in range(len(CHUNKS))
                        ]
                        nmm = kt16 + N8  # matmuls per chunk
                        for k in range(kt16):
                            lhsT = x16_sb[:, k, s * P : (s + 1) * P]
                            for ci, (n0, nw) in enumerate(CHUNKS):
                                nc.tensor.matmul(
                                    pss[ci][:, :nw],
                                    lhsT,
                                    w16_sb[:, k, n0 : n0 + nw],
                                    start=(k == 0),
                                    stop=(k == nmm - 1),
                                )
                        for j in range(N8):
                            lhsT = x8_sb[:, 2 * j : 2 * j + 2, s * P : (s + 1) * P]
                            for ci, (n0, nw) in enumerate(CHUNKS):
                                nc.tensor.matmul(
                                    pss[ci][:, :nw],
                                    lhsT,
                                    w8_sb[:, 2 * j : 2 * j + 2, n0 : n0 + nw],
                                    start=(kt16 + j == 0),
                                    stop=(kt16 + j == nmm - 1),
                                    perf_mode=DR,
                                )
                        for ci, (n0, nw) in enumerate(CHUNKS):
                            if scale != 1.0:
                                nc.vector.tensor_scalar_mul(
                                    o_sb[:, n0 : n0 + nw], pss[ci][:, :nw], 1.0 / scale
                                )
                            else:
                                nc.vector.tensor_copy(
                                    o_sb[:, n0 : n0 + nw], pss[ci][:, :nw]
                                )
                        r0 = b * TB + s * P
                        nc.sync.dma_start(out[r0 : r0 + P, :], o_sb[:])
    nc.compile()
    return nc


def _dequant_f32(qweight, scales, qzeros, g_idx):
    """GPTQ v2 dequant: W = s * (w4 - (z4 + 1)), [in_features, out_features] f32."""
    shifts = np.arange(8, dtype=np.uint32) * 4
    qw = np.ascontiguousarray(qweight).view(np.uint32)
    w4 = (
        ((qw[:, None, :] >> shifts[None, :, None]) & np.uint32(0xF))
        .reshape(-1, qweight.shape[1])
        .astype(np.float32)
    )
    qz = np.ascontiguousarray(qzeros).view(np.uint32)
    z4 = (
        ((qz[:, :, None] >> shifts[None, None, :]) & np.uint32(0xF)).reshape(
            qzeros.shape[0], -1
        )
        + np.uint32(1)
    ).astype(np.float32)
    return scales[g_idx] * (w4 - z4[g_idx])


def prepare_in_maps(inputs):
    """Host-side input prep: dequant, scale, cast, shard. Returns per-core in_maps."""
    x = np.asarray(inputs["x"], dtype=np.float32)
    W = _dequant_f32(
        np.asarray(inputs["qweight"], dtype=np.int32),
        np.asarray(inputs["scales"], dtype=np.float32),
        np.asarray(inputs["qzeros"], dtype=np.int32),
        np.asarray(inputs["g_idx"], dtype=np.int32),
    )
    mdt = np.float16 if MM_DT == "float16" else ml_dtypes.bfloat16
    f8 = ml_dtypes.float8_e4m3fn
    kt16 = KT - 2 * N8
    split = kt16 * P  # contraction rows 0:split are full-precision, rest fp8

    xt = np.ascontiguousarray(x.reshape(-1, IN_F).T)  # [IN_F, TOKENS] f32
    in_maps = [dict() for _ in range(NCORES)]
    if kt16 > 0:
        sx = SX if N8 > 0 else 1.0
        xt16 = np.ascontiguousarray((xt[:split] * sx).astype(mdt))
        for m in in_maps:
            m["xt16"] = xt16
    if N8 > 0:
        xt8 = np.ascontiguousarray(
            np.clip(xt[split:] * SX, -240, 240).astype(f8)
        )
        for m in in_maps:
            m["xt8"] = xt8
    for c in range(NCORES):
        Wc = W[:, c * SHARD : (c + 1) * SHARD]
        if kt16 > 0:
            sw = SW if N8 > 0 else 1.0
            in_maps[c]["w16"] = np.ascontiguousarray((Wc[:split] * sw).astype(mdt))
        if N8 > 0:
            in_maps[c]["w8"] = np.ascontiguousarray(
                np.clip(Wc[split:] * SW, -240, 240).astype(f8)
            )
    return in_maps


def kernel(x, qweight, scales, qzeros, g_idx):
    # NTFF tracing is unavailable under this axon client (antenv.axon_hooks
    # missing); force it off so a stray BASS_TRACE doesn't crash the run.
    os.environ["BASS_NEVER_TRACE"] = "1"
    from concourse.bass_utils import run_bass_kernel_spmd

    x = np.asarray(x, dtype=np.float32)
    in_maps = prepare_in_maps(
        {"x": x, "qweight": qweight, "scales": scales, "qzeros": qzeros, "g_idx": g_idx}
    )

    key = _cfg()
    if _CACHE.get("cfg") != key:
        _CACHE["nc"] = _build_nc()
        _CACHE["cfg"] = key
    nc = _CACHE["nc"]

    res = run_bass_kernel_spmd(nc, in_maps, core_ids=list(range(NCORES)), trace=False)
    _CACHE["last_results"] = res

    out = np.concatenate([res.results[c]["out"] for c in range(NCORES)], axis=1)
    return np.ascontiguousarray(out.reshape(x.shape[0], x.shape[1], OUT_F))

